# revision 23
# baseline (speedup 1.0000x reference)
"""Trainium2 Bass kernel for nn_CustomModel_7378753814838.

Math (reference):
    a = x1.reshape(N,R,F); b = x2.reshape(N,R,F)
    d2[k,n,i,j] = ||a[n,i] - b[n,j] - m_k||^2
    kv = exp(-d2 / (2*sigma_k^2));  out = sum_k w_k * softmax_j(exp(kv))
    with w = softmax(1/sigma_params^2)

Fast path (single surviving kernel k, |sc_k * d2| small -- true for the
staged data, where w is one-hot and sigma ~ -108):
    softmax_j(exp(exp(x))) is invariant to positive scaling of exp(exp(x)),
    and over the actual x = sc*d2 range (|x| < 0.04) a monic quadratic
    (x+h)^2 + g fits exp(exp(x)) to ~1e-6 relative.  Undoing the sc scale,
    p = (d2 + h/sc)^2 + g/sc^2, so the device needs NO transcendentals and
    no per-element scale at all:

    - host: quantize -2(a-m) and b to fp8, transposed to [F, n, i]; compute
      v = rowA + h/sc (split hi/lo bf16) and colB (bf16) from the QUANTIZED
      values so d2 is exact for the quantized inputs
    - PE: per sample, one fp8 128^3 matmul (-2 dot) plus one contraction-3
      bf16 matmul adding v_hi[i] + v_lo[i] + colB[j]; PSUM then holds
      u = d2 + h/sc
    - ACT: per sample one Square: P = u^2 (bf16); samples use one PSUM
      bank each (8 rotating banks) so the PE pipelines 2-matmul chains
    - DVE: per group row-sum of P; per 8 samples a tiny chain
      rec = 1/(S/128 + g') = 128/(S + 128 g'), gr1 = g'*rec - 1; per sample
      one tensor_scalar: delta = P*rec + gr1  (= 128*softmax - 1, bf16)
    - host: out = (delta + 1) / 128

    DMA: input chunks spread across the SP / Activation / Pool queues;
    finals split across DVE and Pool; last output sample exits via the
    Pool queue to shorten the tail.

Sharding: data-parallel over N across 8 cores (16 samples each).
Fallback path (multiple kernels or large |x|): exp/exp via ACT, correct for
any parameters.
"""

import numpy as np

N, R, F, K = 128, 128, 128, 4
NCORES = 8
NP = N // NCORES  # samples per core
GS = 4            # samples per PSUM group (one 2KB psum bank)
NG = NP // GS


def _mld():
    import ml_dtypes

    return ml_dtypes


def _fit_quad(xlo):
    """Least-squares quadratic fit of exp(exp(x)) on [xlo, 0], normalized to
    monic form p(x) = (x+h)^2 + g (softmax is invariant to the scale)."""
    xs = np.linspace(xlo, 0.0, 4001)
    p = np.exp(np.exp(xs))
    M = np.stack([xs * xs, xs, np.ones_like(xs)], 1)
    (a2, a1, a0), *_ = np.linalg.lstsq(M, p, rcond=None)
    h = a1 / (2.0 * a2)
    g = a0 / a2 - h * h
    return float(h), float(g)


def _plan(x1, x2, sigmas, means, sigma_params):
    mld = _mld()
    f8 = mld.float8_e4m3
    bf16 = mld.bfloat16

    sig = np.asarray(sigmas, dtype=np.float64)
    mu = np.asarray(means, dtype=np.float64)
    sp = np.asarray(sigma_params, dtype=np.float64)
    logits = 1.0 / (sp * sp)
    e = np.exp(logits - logits.max())
    w = e / e.sum()
    KS = [k for k in range(K) if w[k] > 1e-4]
    wk = {k: float(w[k] / sum(w[k2] for k2 in KS)) for k in KS}
    SC = {k: float(-1.0 / (2.0 * sig[k] * sig[k])) for k in KS}

    a = x1.reshape(N, R, F).astype(np.float32)
    b = x2.reshape(N, R, F).astype(np.float32)
    Bq = b.astype(f8)
    colB = (Bq.astype(np.float32).astype(np.float64) ** 2).sum(-1)  # [N, R]
    BT = np.ascontiguousarray(Bq.transpose(2, 0, 1))                # [F,N,R]

    plan = {
        "KS": KS, "w": wk, "sc": SC, "BT": BT, "colB": colB,
        "AT": {}, "rowA": {}, "mode": {}, "h": {}, "g": {},
    }
    cb_sqrt_max = np.sqrt(colB).max(axis=1)
    for k in KS:
        A2 = (-2.0 * (a - np.float32(mu[k]))).astype(f8)
        rowA = (A2.astype(np.float32).astype(np.float64) ** 2).sum(-1) / 4.0
        plan["AT"][k] = np.ascontiguousarray(A2.transpose(2, 0, 1))
        plan["rowA"][k] = rowA
        d2ub = ((np.sqrt(rowA).max(axis=1) + cb_sqrt_max) ** 2).max()
        xlo = SC[k] * d2ub
        xfit = -float(2.0 ** np.ceil(np.log2(max(-xlo * 1.05, 1e-4))))
        if -xfit <= 0.35:
            h, g = _fit_quad(xfit)
            plan["mode"][k] = "poly"
            plan["h"][k], plan["g"][k] = h, g
        else:
            plan["mode"][k] = "exp"
            plan["h"][k], plan["g"][k] = 0.0, 0.0
    plan["fast"] = len(KS) == 1 and plan["mode"][KS[0]] == "poly"
    return plan


def _core_inputs_fast(plan, c):
    """Per-core input arrays for the fast path."""
    mld = _mld()
    bf16 = mld.bfloat16
    k = plan["KS"][0]
    s = slice(c * NP, (c + 1) * NP)
    sc, h = plan["sc"][k], plan["h"][k]
    xin = np.empty((F, 2, NP, R), dtype=mld.float8_e4m3)
    xin[:, 0] = plan["AT"][k][:, s, :]
    xin[:, 1] = plan["BT"][:, s, :]
    v = plan["rowA"][k][s] + h / sc                      # [NP, R] f64
    vhi = v.astype(np.float32).astype(bf16)
    vlo = (v - vhi.astype(np.float64)).astype(np.float32).astype(bf16)
    fold = np.zeros((3, 2, NP, R), dtype=bf16)
    fold[0, 0] = vhi
    fold[1, 0] = vlo
    fold[2, 0] = np.ones((NP, R), dtype=bf16)
    fold[0, 1] = np.ones((NP, R), dtype=bf16)
    fold[1, 1] = np.ones((NP, R), dtype=bf16)
    fold[2, 1] = plan["colB"][s].astype(np.float32).astype(bf16)
    return {"xin": np.ascontiguousarray(xin), "fold": np.ascontiguousarray(fold)}


def _build_nc_fast(gq):
    """Fast-path kernel; gq = g/sc^2 is the only baked constant."""
    from contextlib import ExitStack

    import concourse.bacc as bacc
    import concourse.tile as tile
    from concourse import mybir

    f32 = mybir.dt.float32
    bf16 = mybir.dt.bfloat16
    f8 = mybir.dt.float8e4
    ALU = mybir.AluOpType
    ACTF = mybir.ActivationFunctionType

    nc = bacc.Bacc(
        "TRN2",
        target_bir_lowering=False,
        debug=False,
        enable_asserts=False,
        num_devices=NCORES,
    )
    xind = nc.dram_tensor("xin", [F, 2, NP, R], f8, kind="ExternalInput").ap()
    foldd = nc.dram_tensor(
        "fold", [3, 2, NP, R], bf16, kind="ExternalInput"
    ).ap()
    yd = nc.dram_tensor("y", [R, NP, R], bf16, kind="ExternalOutput").ap()

    c_add = float(R * gq)          # S + 128*g'
    c_mul = float(R * gq)          # rec * 128*g'  (then -1)

    with ExitStack() as ctx:
        tc = ctx.enter_context(tile.TileContext(nc))
        singles = ctx.enter_context(tc.tile_pool(name="singles", bufs=1))
        inp = ctx.enter_context(tc.tile_pool(name="inp", bufs=NG))
        pp = ctx.enter_context(tc.tile_pool(name="pp", bufs=NG))
        op = ctx.enter_context(tc.tile_pool(name="op", bufs=NG))
        ps = ctx.enter_context(tc.tile_pool(name="ps", bufs=8, space="PSUM"))

        FT = singles.tile([3, 2, NP, R], bf16)
        nc.gpsimd.dma_start(FT[:], foldd)

        IN = {}
        for g in range(NG):
            IN[g] = inp.tile([F, 2, GS, R], f8, tag=f"in{g}", name=f"in_{g}")
        nc.sync.dma_start(IN[0][:], xind[:, :, 0:GS, :])
        nc.scalar.dma_start(IN[1][:], xind[:, :, GS : 2 * GS, :])
        nc.gpsimd.dma_start(IN[2][:], xind[:, :, 2 * GS : 3 * GS, :])
        nc.sync.dma_start(IN[3][:], xind[:, :, 3 * GS : 4 * GS, :])

        P = {}
        scol = {
            b: singles.tile([R, 2 * GS], f32, name=f"scol{b}") for b in (0, 1)
        }
        r128 = {
            b: singles.tile([R, 2 * GS], f32, name=f"r128{b}") for b in (0, 1)
        }
        gr1 = {
            b: singles.tile([R, 2 * GS], f32, name=f"gr1{b}") for b in (0, 1)
        }
        s2 = {
            b: singles.tile([R, 2 * GS], f32, name=f"s2{b}") for b in (0, 1)
        }

        def tiny(b):
            # rec = 1/(S/128 + g') = 128/(S + 128 g') ; gr1 = g'*rec - 1
            nc.vector.tensor_scalar(
                s2[b][:], scol[b][:], 1.0 / R, float(gq), op0=ALU.mult,
                op1=ALU.add,
            )
            nc.vector.reciprocal_approx_fast(r128[b][:], s2[b][:])
            nc.vector.tensor_scalar(
                gr1[b][:], r128[b][:], float(gq), -1.0, op0=ALU.mult,
                op1=ALU.add,
            )

        def finals(g2):
            b = g2 // 2
            lo2 = GS * (g2 % 2)
            OUTt = op.tile([R, GS, R], bf16, tag=f"OUT{g2}", name=f"OUT_{g2}")
            for q in range(GS):
                rs = r128[b][:, lo2 + q : lo2 + q + 1]
                gs_ = gr1[b][:, lo2 + q : lo2 + q + 1]
                if g2 in (0, 2):
                    eng = nc.vector
                elif g2 == 1:
                    eng = nc.gpsimd
                else:
                    eng = nc.vector if q < 2 else nc.gpsimd
                eng.tensor_scalar(
                    OUTt[:, q, :], P[g2][:, q, :], rs, gs_,
                    op0=ALU.mult, op1=ALU.add,
                )
            s = slice(GS * g2, GS * (g2 + 1))
            if g2 < 3:
                eng = nc.sync if g2 % 2 == 0 else nc.scalar
                eng.dma_start(yd[:, s, :], OUTt[:])
            else:
                nc.scalar.dma_start(
                    yd[:, 12 : NP - 1, :], OUTt[:, 0 : GS - 1, :]
                )
                nc.gpsimd.dma_start(
                    yd[:, NP - 1 : NP, :], OUTt[:, GS - 1 : GS, :]
                )

        for g in range(NG):
            P[g] = pp.tile([R, GS, R], bf16, tag=f"P{g}", name=f"P_{g}")
            b = g // 2
            lo = GS * (g % 2)
            for q in range(GS):
                n = GS * g + q
                bank = ps.tile([R, GS, R], f32, tag="ps", name=f"ps_{n}")
                u = bank[:, 0, :]
                nc.tensor.matmul(
                    u,
                    lhsT=IN[g][:, 0, q, :],
                    rhs=IN[g][:, 1, q, :],
                    start=True,
                    stop=False,
                )
                nc.tensor.matmul(
                    u,
                    lhsT=FT[:, 0, n, :],
                    rhs=FT[:, 1, n, :],
                    start=False,
                    stop=True,
                )
                nc.scalar.activation(P[g][:, q, :], u, ACTF.Square)
            nc.vector.tensor_reduce(
                scol[b][:, lo : lo + GS],
                P[g][:],
                axis=mybir.AxisListType.X,
                op=ALU.add,
            )
            if g == 1:
                tiny(0)
                finals(0)
            elif g == 2:
                finals(1)
            elif g == 3:
                tiny(1)
                finals(2)
                finals(3)

    nc.compile()
    return nc


def _build_nc_general(key):
    """Exp/exp fallback (correct for any parameters); key carries per-kernel
    (mode, sc, h, g, w)."""
    from contextlib import ExitStack

    import concourse.bacc as bacc
    import concourse.tile as tile
    from concourse import mybir

    f32 = mybir.dt.float32
    bf16 = mybir.dt.bfloat16
    f8 = mybir.dt.float8e4
    ALU = mybir.AluOpType
    ACTF = mybir.ActivationFunctionType
    mld = _mld()

    KS, per_k = key
    KS = list(KS)
    per_k = dict(zip(KS, per_k))

    nc = bacc.Bacc(
        "TRN2",
        target_bir_lowering=False,
        debug=False,
        enable_asserts=False,
        num_devices=NCORES,
    )
    ATd = {
        k: nc.dram_tensor(f"at{k}", [F, NP, R], f8, kind="ExternalInput").ap()
        for k in KS
    }
    BTd = nc.dram_tensor("bt", [F, NP, R], f8, kind="ExternalInput").ap()
    CBd = nc.dram_tensor("cb", [1, NP, R], bf16, kind="ExternalInput").ap()
    BIASd = {
        k: nc.dram_tensor(f"bias{k}", [R, NP], f32, kind="ExternalInput").ap()
        for k in KS
    }
    Yd = nc.dram_tensor("y", [R, NP, R], f32, kind="ExternalOutput").ap()
    onesd = nc.inline_tensor(
        np.ones((1, R), dtype=mld.bfloat16), name="ones1"
    ).ap()

    with ExitStack() as ctx:
        tc = ctx.enter_context(tile.TileContext(nc))
        singles = ctx.enter_context(tc.tile_pool(name="singles", bufs=1))
        inp = ctx.enter_context(tc.tile_pool(name="inp", bufs=2 * NG))
        pp = ctx.enter_context(tc.tile_pool(name="pp", bufs=3))
        cols = ctx.enter_context(tc.tile_pool(name="cols", bufs=2 * NG))
        ps = ctx.enter_context(tc.tile_pool(name="ps", bufs=8, space="PSUM"))

        ones = singles.tile([1, R], bf16)
        nc.sync.dma_start(ones[:], onesd)
        CBt = singles.tile([1, NP, R], bf16)
        nc.sync.dma_start(CBt[:], CBd)
        BIASt = {
            k: singles.tile([R, NP], f32, name=f"biast{k}") for k in KS
        }
        for k in KS:
            nc.sync.dma_start(BIASt[k][:], BIASd[k])

        AT = {}
        BT = {}
        for g in range(NG):
            s = slice(GS * g, GS * (g + 1))
            for k in KS:
                AT[(k, g)] = inp.tile(
                    [F, GS, R], f8, tag=f"at{k}{g % 2}", name=f"at{k}_{g}"
                )
                nc.sync.dma_start(AT[(k, g)][:], ATd[k][:, s, :])
            BT[g] = inp.tile([F, GS, R], f8, tag=f"bt{g % 2}", name=f"bt_{g}")
            nc.scalar.dma_start(BT[g][:], BTd[:, s, :])

        OUTacc = singles.tile([R, NP, R], f32)

        for g in range(NG):
            s = slice(GS * g, GS * (g + 1))
            for ki, k in enumerate(KS):
                mode, sc, h, gq, wkk = per_k[k]
                pst = ps.tile([R, GS, R], f32, tag="ps")
                for q in range(GS):
                    nc.tensor.matmul(
                        pst[:, q, :],
                        lhsT=AT[(k, g)][:, q, :],
                        rhs=BT[g][:, q, :],
                        start=(q == 0),
                        stop=False,
                    )
                nc.tensor.matmul(
                    pst[:, :, :],
                    lhsT=ones[:],
                    rhs=CBt[:, s, :],
                    start=False,
                    stop=True,
                )
                scol = cols.tile([R, GS], f32, tag="scol")
                KV = pp.tile([R, GS, R], f32, tag="KV")
                E = pp.tile([R, GS, R], f32, tag="E")
                for q in range(GS):
                    n = GS * g + q
                    nc.scalar.activation(
                        KV[:, q, :],
                        pst[:, q, :],
                        ACTF.Exp,
                        bias=BIASt[k][:, n : n + 1],
                        scale=sc,
                    )
                    nc.scalar.activation(
                        E[:, q, :],
                        KV[:, q, :],
                        ACTF.Exp,
                        accum_out=scol[:, q : q + 1],
                    )
                rcol = cols.tile([R, GS], f32, tag="rcol")
                nc.vector.reciprocal_approx_fast(rcol[:], scol[:])
                if wkk != 1.0:
                    nc.vector.tensor_scalar(
                        rcol[:], rcol[:], float(wkk), None, op0=ALU.mult
                    )
                for q in range(GS):
                    n = GS * g + q
                    if ki == 0:
                        nc.vector.tensor_scalar(
                            OUTacc[:, n, :],
                            E[:, q, :],
                            rcol[:, q : q + 1],
                            None,
                            op0=ALU.mult,
                        )
                    else:
                        nc.vector.scalar_tensor_tensor(
                            OUTacc[:, n, :],
                            E[:, q, :],
                            rcol[:, q : q + 1],
                            OUTacc[:, n, :],
                            op0=ALU.mult,
                            op1=ALU.add,
                        )
            eng = nc.sync if g % 2 == 0 else nc.scalar
            eng.dma_start(Yd[:, s, :], OUTacc[:, s, :])

    nc.compile()
    return nc


_CACHE = {}


def run(x1, x2, sigmas, means, sigma_params, trace=False, **rk):
    from concourse.bass_utils import run_bass_kernel_spmd

    x1 = np.ascontiguousarray(x1, dtype=np.float32)
    x2 = np.ascontiguousarray(x2, dtype=np.float32)
    plan = _plan(x1, x2, sigmas, means, sigma_params)
    KS = plan["KS"]

    if plan["fast"]:
        k = KS[0]
        gq = plan["g"][k] / (plan["sc"][k] ** 2)
        key = ("fast", float(gq))
        if key not in _CACHE:
            _CACHE[key] = _build_nc_fast(float(gq))
        nc = _CACHE[key]
        in_maps = [_core_inputs_fast(plan, c) for c in range(NCORES)]
        res = run_bass_kernel_spmd(
            nc, in_maps, core_ids=list(range(NCORES)), trace=trace, **rk
        )
        out = np.concatenate(
            [
                (
                    (np.asarray(r["y"]).astype(np.float32) + 1.0)
                    * np.float32(1.0 / R)
                ).transpose(1, 0, 2)
                for r in res.results
            ],
            axis=0,
        )
        return out, res

    key = (
        tuple(KS),
        tuple(
            (plan["mode"][k], plan["sc"][k], plan["h"][k], plan["g"][k],
             plan["w"][k])
            for k in KS
        ),
    )
    if key not in _CACHE:
        _CACHE[key] = _build_nc_general(key)
    nc = _CACHE[key]
    in_maps = []
    for c in range(NCORES):
        s = slice(c * NP, (c + 1) * NP)
        m = {
            "bt": np.ascontiguousarray(plan["BT"][:, s, :]),
            "cb": np.ascontiguousarray(
                plan["colB"][s].astype(np.float32).astype(_mld().bfloat16)
            )[None],
        }
        for k in KS:
            m[f"at{k}"] = np.ascontiguousarray(plan["AT"][k][:, s, :])
            bias = plan["sc"][k] * plan["rowA"][k][s]  # [NP, R]
            m[f"bias{k}"] = np.ascontiguousarray(
                bias.astype(np.float32).transpose()
            )
        in_maps.append(m)
    res = run_bass_kernel_spmd(
        nc, in_maps, core_ids=list(range(NCORES)), trace=trace, **rk
    )
    out = np.concatenate(
        [np.asarray(r["y"]).astype(np.float32).transpose(1, 0, 2)
         for r in res.results],
        axis=0,
    )
    return out, res


def kernel(x1, x2, sigmas, means, sigma_params):
    out, _ = run(x1, x2, sigmas, means, sigma_params, trace=False)
    return out


# revision 24
# speedup vs baseline: 1.0440x; 1.0440x over previous
"""Trainium2 Bass kernel for nn_CustomModel_7378753814838.

Math (reference):
    a = x1.reshape(N,R,F); b = x2.reshape(N,R,F)
    d2[k,n,i,j] = ||a[n,i] - b[n,j] - m_k||^2
    kv = exp(-d2 / (2*sigma_k^2));  out = sum_k w_k * softmax_j(exp(kv))
    with w = softmax(1/sigma_params^2)

Fast path (single surviving kernel k, |sc_k * d2| small -- true for the
staged data, where w is one-hot and sigma ~ -108):
    softmax_j(exp(exp(x))) is invariant to positive scaling of exp(exp(x)),
    and over the actual x = sc*d2 range (|x| < 0.04) a monic quadratic
    (x+h)^2 + g fits exp(exp(x)) to ~1e-6 relative.  Undoing the sc scale,
    p = (d2 + h/sc)^2 + g/sc^2, so the device needs NO transcendentals and
    no per-element scale at all:

    - host: quantize -2(a-m) and b to fp8, transposed to [F, n, i]; compute
      v = rowA + h/sc (split hi/lo bf16) and colB (bf16) from the QUANTIZED
      values so d2 is exact for the quantized inputs
    - PE: per sample, one fp8 128^3 matmul (-2 dot) plus one contraction-3
      bf16 matmul adding v_hi[i] + v_lo[i] + colB[j]; PSUM then holds
      u = d2 + h/sc
    - ACT: per sample one Square: P = u^2 (bf16); samples use one PSUM
      bank each (8 rotating banks) so the PE pipelines 2-matmul chains
    - DVE: per group row-sum of P; per 8 samples a tiny chain
      rec = 1/(S/128 + g') = 128/(S + 128 g'), gr1 = g'*rec - 1; per sample
      one tensor_scalar: delta = P*rec + gr1  (= 128*softmax - 1, bf16)
    - host: out = (delta + 1) / 128

    DMA: input chunks spread across the SP / Activation / Pool queues;
    finals split across DVE and Pool; last output sample exits via the
    Pool queue to shorten the tail.

Sharding: data-parallel over N across 8 cores (16 samples each).
Fallback path (multiple kernels or large |x|): exp/exp via ACT, correct for
any parameters.
"""

import numpy as np

N, R, F, K = 128, 128, 128, 4
NCORES = 8
NP = N // NCORES  # samples per core
GS = 4            # samples per PSUM group (one 2KB psum bank)
NG = NP // GS


def _mld():
    import ml_dtypes

    return ml_dtypes


def _fit_quad(xlo):
    """Least-squares quadratic fit of exp(exp(x)) on [xlo, 0], normalized to
    monic form p(x) = (x+h)^2 + g (softmax is invariant to the scale)."""
    xs = np.linspace(xlo, 0.0, 4001)
    p = np.exp(np.exp(xs))
    M = np.stack([xs * xs, xs, np.ones_like(xs)], 1)
    (a2, a1, a0), *_ = np.linalg.lstsq(M, p, rcond=None)
    h = a1 / (2.0 * a2)
    g = a0 / a2 - h * h
    return float(h), float(g)


def _plan(x1, x2, sigmas, means, sigma_params):
    mld = _mld()
    f8 = mld.float8_e4m3
    bf16 = mld.bfloat16

    sig = np.asarray(sigmas, dtype=np.float64)
    mu = np.asarray(means, dtype=np.float64)
    sp = np.asarray(sigma_params, dtype=np.float64)
    logits = 1.0 / (sp * sp)
    e = np.exp(logits - logits.max())
    w = e / e.sum()
    KS = [k for k in range(K) if w[k] > 1e-4]
    wk = {k: float(w[k] / sum(w[k2] for k2 in KS)) for k in KS}
    SC = {k: float(-1.0 / (2.0 * sig[k] * sig[k])) for k in KS}

    a = x1.reshape(N, R, F).astype(np.float32)
    b = x2.reshape(N, R, F).astype(np.float32)
    Bq = b.astype(f8)
    colB = (Bq.astype(np.float32).astype(np.float64) ** 2).sum(-1)  # [N, R]
    BT = np.ascontiguousarray(Bq.transpose(2, 0, 1))                # [F,N,R]

    plan = {
        "KS": KS, "w": wk, "sc": SC, "BT": BT, "colB": colB,
        "AT": {}, "rowA": {}, "mode": {}, "h": {}, "g": {},
    }
    cb_sqrt_max = np.sqrt(colB).max(axis=1)
    for k in KS:
        A2 = (-2.0 * (a - np.float32(mu[k]))).astype(f8)
        rowA = (A2.astype(np.float32).astype(np.float64) ** 2).sum(-1) / 4.0
        plan["AT"][k] = np.ascontiguousarray(A2.transpose(2, 0, 1))
        plan["rowA"][k] = rowA
        d2ub = ((np.sqrt(rowA).max(axis=1) + cb_sqrt_max) ** 2).max()
        xlo = SC[k] * d2ub
        xfit = -float(2.0 ** np.ceil(np.log2(max(-xlo * 1.05, 1e-4))))
        if -xfit <= 0.35:
            h, g = _fit_quad(xfit)
            plan["mode"][k] = "poly"
            plan["h"][k], plan["g"][k] = h, g
        else:
            plan["mode"][k] = "exp"
            plan["h"][k], plan["g"][k] = 0.0, 0.0
    plan["fast"] = len(KS) == 1 and plan["mode"][KS[0]] == "poly"
    return plan


def _core_inputs_fast(plan, c):
    """Per-core input arrays for the fast path."""
    mld = _mld()
    bf16 = mld.bfloat16
    k = plan["KS"][0]
    s = slice(c * NP, (c + 1) * NP)
    sc, h = plan["sc"][k], plan["h"][k]
    xin = np.empty((F, 2, NP, R), dtype=mld.float8_e4m3)
    xin[:, 0] = plan["AT"][k][:, s, :]
    xin[:, 1] = plan["BT"][:, s, :]
    v = plan["rowA"][k][s] + h / sc                      # [NP, R] f64
    vhi = v.astype(np.float32).astype(bf16)
    vlo = (v - vhi.astype(np.float64)).astype(np.float32).astype(bf16)
    fold = np.zeros((3, 2, NP, R), dtype=bf16)
    fold[0, 0] = vhi
    fold[1, 0] = vlo
    fold[2, 0] = np.ones((NP, R), dtype=bf16)
    fold[0, 1] = np.ones((NP, R), dtype=bf16)
    fold[1, 1] = np.ones((NP, R), dtype=bf16)
    fold[2, 1] = plan["colB"][s].astype(np.float32).astype(bf16)
    return {"xin": np.ascontiguousarray(xin), "fold": np.ascontiguousarray(fold)}


def _build_nc_fast(gq):
    """Fast-path kernel; gq = g/sc^2 is the only baked constant."""
    from contextlib import ExitStack

    import concourse.bacc as bacc
    import concourse.tile as tile
    from concourse import mybir

    f32 = mybir.dt.float32
    bf16 = mybir.dt.bfloat16
    f8 = mybir.dt.float8e4
    ALU = mybir.AluOpType
    ACTF = mybir.ActivationFunctionType

    nc = bacc.Bacc(
        "TRN2",
        target_bir_lowering=False,
        debug=False,
        enable_asserts=False,
        num_devices=NCORES,
    )
    xind = nc.dram_tensor("xin", [F, 2, NP, R], f8, kind="ExternalInput").ap()
    foldd = nc.dram_tensor(
        "fold", [3, 2, NP, R], bf16, kind="ExternalInput"
    ).ap()
    yd = nc.dram_tensor("y", [R, NP, R], bf16, kind="ExternalOutput").ap()

    c_add = float(R * gq)          # S + 128*g'
    c_mul = float(R * gq)          # rec * 128*g'  (then -1)

    with ExitStack() as ctx:
        tc = ctx.enter_context(tile.TileContext(nc))
        singles = ctx.enter_context(tc.tile_pool(name="singles", bufs=1))
        inp = ctx.enter_context(tc.tile_pool(name="inp", bufs=NG))
        pp = ctx.enter_context(tc.tile_pool(name="pp", bufs=NG))
        op = ctx.enter_context(tc.tile_pool(name="op", bufs=NG))
        ps = ctx.enter_context(tc.tile_pool(name="ps", bufs=8, space="PSUM"))

        FT = singles.tile([3, 2, NP, R], bf16)
        nc.gpsimd.dma_start(FT[:], foldd)

        IN = {}
        for g in range(NG):
            IN[g] = inp.tile([F, 2, GS, R], f8, tag=f"in{g}", name=f"in_{g}")
        nc.sync.dma_start(IN[0][:], xind[:, :, 0:GS, :])
        nc.scalar.dma_start(IN[1][:], xind[:, :, GS : 2 * GS, :])
        nc.gpsimd.dma_start(IN[2][:], xind[:, :, 2 * GS : 3 * GS, :])
        nc.sync.dma_start(IN[3][:], xind[:, :, 3 * GS : 4 * GS, :])

        P = {}
        scolt = {
            g: singles.tile([R, GS], f32, name=f"scol{g}") for g in range(NG)
        }
        s2t = {
            g: singles.tile([R, GS], f32, name=f"s2_{g}") for g in range(NG)
        }
        rec = {
            g: singles.tile([R, GS], f32, name=f"rec{g}") for g in range(NG)
        }
        gr = {
            g: singles.tile([R, GS], f32, name=f"gr{g}") for g in range(NG)
        }

        for g in range(NG):
            P[g] = pp.tile([R, GS, R], bf16, tag=f"P{g}", name=f"P_{g}")
            for q in range(GS):
                n = GS * g + q
                bank = ps.tile([R, GS, R], f32, tag="ps", name=f"ps_{n}")
                u = bank[:, 0, :]
                nc.tensor.matmul(
                    u,
                    lhsT=IN[g][:, 0, q, :],
                    rhs=IN[g][:, 1, q, :],
                    start=True,
                    stop=False,
                )
                nc.tensor.matmul(
                    u,
                    lhsT=FT[:, 0, n, :],
                    rhs=FT[:, 1, n, :],
                    start=False,
                    stop=True,
                )
                if g == NG - 1:
                    # last group: fuse the row-sum into the Square so the
                    # tail does not wait for a separate reduce
                    nc.scalar.activation(
                        P[g][:, q, :], u, ACTF.Square,
                        accum_out=scolt[g][:, q : q + 1],
                    )
                else:
                    nc.scalar.activation(P[g][:, q, :], u, ACTF.Square)
            if g < NG - 1:
                nc.vector.tensor_reduce(
                    scolt[g][:],
                    P[g][:],
                    axis=mybir.AxisListType.X,
                    op=ALU.add,
                )
            # per-group tiny chain (DVE):
            # rec = 1/(S/128 + g') = 128/(S + 128 g') ; gr = g'*rec - 1
            nc.vector.tensor_scalar(
                s2t[g][:], scolt[g][:], 1.0 / R, float(gq), op0=ALU.mult,
                op1=ALU.add,
            )
            nc.vector.reciprocal_approx_fast(rec[g][:], s2t[g][:])
            nc.vector.tensor_scalar(
                gr[g][:], rec[g][:], float(gq), -1.0, op0=ALU.mult,
                op1=ALU.add,
            )
            OUTt = op.tile([R, GS, R], bf16, tag=f"OUT{g}", name=f"OUT_{g}")
            for q in range(GS):
                eng = nc.vector if q < 2 else nc.gpsimd
                eng.tensor_scalar(
                    OUTt[:, q, :], P[g][:, q, :],
                    rec[g][:, q : q + 1], gr[g][:, q : q + 1],
                    op0=ALU.mult, op1=ALU.add,
                )
            s = slice(GS * g, GS * (g + 1))
            if g < NG - 1:
                nc.sync.dma_start(yd[:, s, :], OUTt[:])
            else:
                nc.scalar.dma_start(yd[:, 12:14, :], OUTt[:, 0:2, :])
                nc.gpsimd.dma_start(yd[:, 14:NP, :], OUTt[:, 2:GS, :])

    nc.compile()
    return nc


def _build_nc_general(key):
    """Exp/exp fallback (correct for any parameters); key carries per-kernel
    (mode, sc, h, g, w)."""
    from contextlib import ExitStack

    import concourse.bacc as bacc
    import concourse.tile as tile
    from concourse import mybir

    f32 = mybir.dt.float32
    bf16 = mybir.dt.bfloat16
    f8 = mybir.dt.float8e4
    ALU = mybir.AluOpType
    ACTF = mybir.ActivationFunctionType
    mld = _mld()

    KS, per_k = key
    KS = list(KS)
    per_k = dict(zip(KS, per_k))

    nc = bacc.Bacc(
        "TRN2",
        target_bir_lowering=False,
        debug=False,
        enable_asserts=False,
        num_devices=NCORES,
    )
    ATd = {
        k: nc.dram_tensor(f"at{k}", [F, NP, R], f8, kind="ExternalInput").ap()
        for k in KS
    }
    BTd = nc.dram_tensor("bt", [F, NP, R], f8, kind="ExternalInput").ap()
    CBd = nc.dram_tensor("cb", [1, NP, R], bf16, kind="ExternalInput").ap()
    BIASd = {
        k: nc.dram_tensor(f"bias{k}", [R, NP], f32, kind="ExternalInput").ap()
        for k in KS
    }
    Yd = nc.dram_tensor("y", [R, NP, R], f32, kind="ExternalOutput").ap()
    onesd = nc.inline_tensor(
        np.ones((1, R), dtype=mld.bfloat16), name="ones1"
    ).ap()

    with ExitStack() as ctx:
        tc = ctx.enter_context(tile.TileContext(nc))
        singles = ctx.enter_context(tc.tile_pool(name="singles", bufs=1))
        inp = ctx.enter_context(tc.tile_pool(name="inp", bufs=2 * NG))
        pp = ctx.enter_context(tc.tile_pool(name="pp", bufs=3))
        cols = ctx.enter_context(tc.tile_pool(name="cols", bufs=2 * NG))
        ps = ctx.enter_context(tc.tile_pool(name="ps", bufs=8, space="PSUM"))

        ones = singles.tile([1, R], bf16)
        nc.sync.dma_start(ones[:], onesd)
        CBt = singles.tile([1, NP, R], bf16)
        nc.sync.dma_start(CBt[:], CBd)
        BIASt = {
            k: singles.tile([R, NP], f32, name=f"biast{k}") for k in KS
        }
        for k in KS:
            nc.sync.dma_start(BIASt[k][:], BIASd[k])

        AT = {}
        BT = {}
        for g in range(NG):
            s = slice(GS * g, GS * (g + 1))
            for k in KS:
                AT[(k, g)] = inp.tile(
                    [F, GS, R], f8, tag=f"at{k}{g % 2}", name=f"at{k}_{g}"
                )
                nc.sync.dma_start(AT[(k, g)][:], ATd[k][:, s, :])
            BT[g] = inp.tile([F, GS, R], f8, tag=f"bt{g % 2}", name=f"bt_{g}")
            nc.scalar.dma_start(BT[g][:], BTd[:, s, :])

        OUTacc = singles.tile([R, NP, R], f32)

        for g in range(NG):
            s = slice(GS * g, GS * (g + 1))
            for ki, k in enumerate(KS):
                mode, sc, h, gq, wkk = per_k[k]
                pst = ps.tile([R, GS, R], f32, tag="ps")
                for q in range(GS):
                    nc.tensor.matmul(
                        pst[:, q, :],
                        lhsT=AT[(k, g)][:, q, :],
                        rhs=BT[g][:, q, :],
                        start=(q == 0),
                        stop=False,
                    )
                nc.tensor.matmul(
                    pst[:, :, :],
                    lhsT=ones[:],
                    rhs=CBt[:, s, :],
                    start=False,
                    stop=True,
                )
                scol = cols.tile([R, GS], f32, tag="scol")
                KV = pp.tile([R, GS, R], f32, tag="KV")
                E = pp.tile([R, GS, R], f32, tag="E")
                for q in range(GS):
                    n = GS * g + q
                    nc.scalar.activation(
                        KV[:, q, :],
                        pst[:, q, :],
                        ACTF.Exp,
                        bias=BIASt[k][:, n : n + 1],
                        scale=sc,
                    )
                    nc.scalar.activation(
                        E[:, q, :],
                        KV[:, q, :],
                        ACTF.Exp,
                        accum_out=scol[:, q : q + 1],
                    )
                rcol = cols.tile([R, GS], f32, tag="rcol")
                nc.vector.reciprocal_approx_fast(rcol[:], scol[:])
                if wkk != 1.0:
                    nc.vector.tensor_scalar(
                        rcol[:], rcol[:], float(wkk), None, op0=ALU.mult
                    )
                for q in range(GS):
                    n = GS * g + q
                    if ki == 0:
                        nc.vector.tensor_scalar(
                            OUTacc[:, n, :],
                            E[:, q, :],
                            rcol[:, q : q + 1],
                            None,
                            op0=ALU.mult,
                        )
                    else:
                        nc.vector.scalar_tensor_tensor(
                            OUTacc[:, n, :],
                            E[:, q, :],
                            rcol[:, q : q + 1],
                            OUTacc[:, n, :],
                            op0=ALU.mult,
                            op1=ALU.add,
                        )
            eng = nc.sync if g % 2 == 0 else nc.scalar
            eng.dma_start(Yd[:, s, :], OUTacc[:, s, :])

    nc.compile()
    return nc


_CACHE = {}


def run(x1, x2, sigmas, means, sigma_params, trace=False, **rk):
    from concourse.bass_utils import run_bass_kernel_spmd

    x1 = np.ascontiguousarray(x1, dtype=np.float32)
    x2 = np.ascontiguousarray(x2, dtype=np.float32)
    plan = _plan(x1, x2, sigmas, means, sigma_params)
    KS = plan["KS"]

    if plan["fast"]:
        k = KS[0]
        gq = plan["g"][k] / (plan["sc"][k] ** 2)
        key = ("fast", float(gq))
        if key not in _CACHE:
            _CACHE[key] = _build_nc_fast(float(gq))
        nc = _CACHE[key]
        in_maps = [_core_inputs_fast(plan, c) for c in range(NCORES)]
        res = run_bass_kernel_spmd(
            nc, in_maps, core_ids=list(range(NCORES)), trace=trace, **rk
        )
        out = np.concatenate(
            [
                (
                    (np.asarray(r["y"]).astype(np.float32) + 1.0)
                    * np.float32(1.0 / R)
                ).transpose(1, 0, 2)
                for r in res.results
            ],
            axis=0,
        )
        return out, res

    key = (
        tuple(KS),
        tuple(
            (plan["mode"][k], plan["sc"][k], plan["h"][k], plan["g"][k],
             plan["w"][k])
            for k in KS
        ),
    )
    if key not in _CACHE:
        _CACHE[key] = _build_nc_general(key)
    nc = _CACHE[key]
    in_maps = []
    for c in range(NCORES):
        s = slice(c * NP, (c + 1) * NP)
        m = {
            "bt": np.ascontiguousarray(plan["BT"][:, s, :]),
            "cb": np.ascontiguousarray(
                plan["colB"][s].astype(np.float32).astype(_mld().bfloat16)
            )[None],
        }
        for k in KS:
            m[f"at{k}"] = np.ascontiguousarray(plan["AT"][k][:, s, :])
            bias = plan["sc"][k] * plan["rowA"][k][s]  # [NP, R]
            m[f"bias{k}"] = np.ascontiguousarray(
                bias.astype(np.float32).transpose()
            )
        in_maps.append(m)
    res = run_bass_kernel_spmd(
        nc, in_maps, core_ids=list(range(NCORES)), trace=trace, **rk
    )
    out = np.concatenate(
        [np.asarray(r["y"]).astype(np.float32).transpose(1, 0, 2)
         for r in res.results],
        axis=0,
    )
    return out, res


def kernel(x1, x2, sigmas, means, sigma_params):
    out, _ = run(x1, x2, sigmas, means, sigma_params, trace=False)
    return out


# revision 25
# speedup vs baseline: 1.0719x; 1.0267x over previous
"""Trainium2 Bass kernel for nn_CustomModel_7378753814838.

Math (reference):
    a = x1.reshape(N,R,F); b = x2.reshape(N,R,F)
    d2[k,n,i,j] = ||a[n,i] - b[n,j] - m_k||^2
    kv = exp(-d2 / (2*sigma_k^2));  out = sum_k w_k * softmax_j(exp(kv))
    with w = softmax(1/sigma_params^2)

Fast path (single surviving kernel k, |sc_k * d2| small -- true for the
staged data, where w is one-hot and sigma ~ -108):
    softmax_j(exp(exp(x))) is invariant to positive scaling of exp(exp(x)),
    and over the actual x = sc*d2 range (|x| < 0.04) a monic quadratic
    (x+h)^2 + g fits exp(exp(x)) to ~1e-6 relative.  Undoing the sc scale,
    p = (d2 + h/sc)^2 + g/sc^2, so the device needs NO transcendentals and
    no per-element scale at all:

    - host: quantize -2(a-m) and b to fp8, transposed to [F, n, i]; compute
      v = rowA + h/sc (split hi/lo bf16) and colB (bf16) from the QUANTIZED
      values so d2 is exact for the quantized inputs
    - PE: per sample, one fp8 128^3 matmul (-2 dot) plus one contraction-3
      bf16 matmul adding v_hi[i] + v_lo[i] + colB[j]; PSUM then holds
      u = d2 + h/sc
    - ACT: per sample one Square: P = u^2 (bf16); samples use one PSUM
      bank each (8 rotating banks) so the PE pipelines 2-matmul chains
    - DVE: per group row-sum of P; per 8 samples a tiny chain
      rec = 1/(S/128 + g') = 128/(S + 128 g'), gr1 = g'*rec - 1; per sample
      one tensor_scalar: delta = P*rec + gr1  (= 128*softmax - 1, bf16)
    - host: out = (delta + 1) / 128

    DMA: input chunks spread across the SP / Activation / Pool queues;
    finals split across DVE and Pool; last output sample exits via the
    Pool queue to shorten the tail.

Sharding: data-parallel over N across 8 cores (16 samples each).
Fallback path (multiple kernels or large |x|): exp/exp via ACT, correct for
any parameters.
"""

import numpy as np

N, R, F, K = 128, 128, 128, 4
NCORES = 8
NP = N // NCORES  # samples per core
GS = 4            # samples per PSUM group (one 2KB psum bank)
NG = NP // GS


def _mld():
    import ml_dtypes

    return ml_dtypes


def _fit_quad(xlo):
    """Least-squares quadratic fit of exp(exp(x)) on [xlo, 0], normalized to
    monic form p(x) = (x+h)^2 + g (softmax is invariant to the scale)."""
    xs = np.linspace(xlo, 0.0, 4001)
    p = np.exp(np.exp(xs))
    M = np.stack([xs * xs, xs, np.ones_like(xs)], 1)
    (a2, a1, a0), *_ = np.linalg.lstsq(M, p, rcond=None)
    h = a1 / (2.0 * a2)
    g = a0 / a2 - h * h
    return float(h), float(g)


def _plan(x1, x2, sigmas, means, sigma_params):
    mld = _mld()
    f8 = mld.float8_e4m3
    bf16 = mld.bfloat16

    sig = np.asarray(sigmas, dtype=np.float64)
    mu = np.asarray(means, dtype=np.float64)
    sp = np.asarray(sigma_params, dtype=np.float64)
    logits = 1.0 / (sp * sp)
    e = np.exp(logits - logits.max())
    w = e / e.sum()
    KS = [k for k in range(K) if w[k] > 1e-4]
    wk = {k: float(w[k] / sum(w[k2] for k2 in KS)) for k in KS}
    SC = {k: float(-1.0 / (2.0 * sig[k] * sig[k])) for k in KS}

    a = x1.reshape(N, R, F).astype(np.float32)
    b = x2.reshape(N, R, F).astype(np.float32)
    Bq = b.astype(f8)
    colB = (Bq.astype(np.float32).astype(np.float64) ** 2).sum(-1)  # [N, R]
    BT = np.ascontiguousarray(Bq.transpose(2, 0, 1))                # [F,N,R]

    plan = {
        "KS": KS, "w": wk, "sc": SC, "BT": BT, "colB": colB,
        "AT": {}, "rowA": {}, "mode": {}, "h": {}, "g": {},
    }
    cb_sqrt_max = np.sqrt(colB).max(axis=1)
    for k in KS:
        A2 = (-2.0 * (a - np.float32(mu[k]))).astype(f8)
        rowA = (A2.astype(np.float32).astype(np.float64) ** 2).sum(-1) / 4.0
        plan["AT"][k] = np.ascontiguousarray(A2.transpose(2, 0, 1))
        plan["rowA"][k] = rowA
        d2ub = ((np.sqrt(rowA).max(axis=1) + cb_sqrt_max) ** 2).max()
        xlo = SC[k] * d2ub
        xfit = -float(2.0 ** np.ceil(np.log2(max(-xlo * 1.05, 1e-4))))
        if -xfit <= 0.35:
            h, g = _fit_quad(xfit)
            plan["mode"][k] = "poly"
            plan["h"][k], plan["g"][k] = h, g
        else:
            plan["mode"][k] = "exp"
            plan["h"][k], plan["g"][k] = 0.0, 0.0
    plan["fast"] = len(KS) == 1 and plan["mode"][KS[0]] == "poly"
    return plan


def _core_inputs_fast(plan, c):
    """Per-core input arrays for the fast path."""
    mld = _mld()
    bf16 = mld.bfloat16
    k = plan["KS"][0]
    s = slice(c * NP, (c + 1) * NP)
    sc, h = plan["sc"][k], plan["h"][k]
    xin = np.empty((F, 2, NP, R), dtype=mld.float8_e4m3)
    xin[:, 0] = plan["AT"][k][:, s, :]
    xin[:, 1] = plan["BT"][:, s, :]
    v = plan["rowA"][k][s] + h / sc                      # [NP, R] f64
    vhi = v.astype(np.float32).astype(bf16)
    vlo = (v - vhi.astype(np.float64)).astype(np.float32).astype(bf16)
    fold = np.zeros((3, 2, NP, R), dtype=bf16)
    fold[0, 0] = vhi
    fold[1, 0] = vlo
    fold[2, 0] = np.ones((NP, R), dtype=bf16)
    fold[0, 1] = np.ones((NP, R), dtype=bf16)
    fold[1, 1] = np.ones((NP, R), dtype=bf16)
    fold[2, 1] = plan["colB"][s].astype(np.float32).astype(bf16)
    return {"xin": np.ascontiguousarray(xin), "fold": np.ascontiguousarray(fold)}


def _build_nc_fast(gq):
    """Fast-path kernel; gq = g/sc^2 is the only baked constant."""
    from contextlib import ExitStack

    import concourse.bacc as bacc
    import concourse.tile as tile
    from concourse import mybir

    f32 = mybir.dt.float32
    bf16 = mybir.dt.bfloat16
    f8 = mybir.dt.float8e4
    ALU = mybir.AluOpType
    ACTF = mybir.ActivationFunctionType

    nc = bacc.Bacc(
        "TRN2",
        target_bir_lowering=False,
        debug=False,
        enable_asserts=False,
        num_devices=NCORES,
    )
    xind = nc.dram_tensor("xin", [F, 2, NP, R], f8, kind="ExternalInput").ap()
    foldd = nc.dram_tensor(
        "fold", [3, 2, NP, R], bf16, kind="ExternalInput"
    ).ap()
    yd = nc.dram_tensor("y", [R, NP, R], bf16, kind="ExternalOutput").ap()

    c_add = float(R * gq)          # S + 128*g'
    c_mul = float(R * gq)          # rec * 128*g'  (then -1)

    with ExitStack() as ctx:
        tc = ctx.enter_context(tile.TileContext(nc))
        singles = ctx.enter_context(tc.tile_pool(name="singles", bufs=1))
        inp = ctx.enter_context(tc.tile_pool(name="inp", bufs=NG))
        pp = ctx.enter_context(tc.tile_pool(name="pp", bufs=NG))
        op = ctx.enter_context(tc.tile_pool(name="op", bufs=NG))
        ps = ctx.enter_context(tc.tile_pool(name="ps", bufs=8, space="PSUM"))

        FT = singles.tile([3, 2, NP, R], bf16)
        nc.gpsimd.dma_start(FT[:], foldd)

        IN = {}
        for g in range(NG):
            IN[g] = inp.tile([F, 2, GS, R], f8, tag=f"in{g}", name=f"in_{g}")
        nc.sync.dma_start(IN[0][:], xind[:, :, 0:GS, :])
        nc.scalar.dma_start(IN[1][:], xind[:, :, GS : 2 * GS, :])
        nc.gpsimd.dma_start(IN[2][:], xind[:, :, 2 * GS : 3 * GS, :])
        nc.sync.dma_start(IN[3][:], xind[:, :, 3 * GS : 4 * GS, :])

        P = {}
        scolt = {
            g: singles.tile([R, GS], f32, name=f"scol{g}") for g in range(NG)
        }
        s2t = {
            g: singles.tile([R, GS], f32, name=f"s2_{g}") for g in range(NG)
        }
        rec = {
            g: singles.tile([R, GS], f32, name=f"rec{g}") for g in range(NG)
        }
        gr = {
            g: singles.tile([R, GS], f32, name=f"gr{g}") for g in range(NG)
        }

        def tiny(g):
            # rec = 1/(S/128 + g') = 128/(S + 128 g') ; gr = g'*rec - 1
            nc.vector.tensor_scalar(
                s2t[g][:], scolt[g][:], 1.0 / R, float(gq), op0=ALU.mult,
                op1=ALU.add,
            )
            nc.vector.reciprocal_approx_fast(rec[g][:], s2t[g][:])
            nc.vector.tensor_scalar(
                gr[g][:], rec[g][:], float(gq), -1.0, op0=ALU.mult,
                op1=ALU.add,
            )

        OUTT = {}

        def finals(g2, engs):
            OUTt = op.tile([R, GS, R], bf16, tag=f"OUT{g2}", name=f"OUT_{g2}")
            OUTT[g2] = OUTt
            for q in range(GS):
                rs = rec[g2][:, q : q + 1]
                gs_ = gr[g2][:, q : q + 1]
                eng = engs[q]
                if eng == "act":
                    nc.scalar.activation(
                        OUTt[:, q, :], P[g2][:, q, :], ACTF.Identity,
                        bias=gs_, scale=rs,
                    )
                elif eng == "pool":
                    nc.gpsimd.tensor_scalar(
                        OUTt[:, q, :], P[g2][:, q, :], rs, gs_,
                        op0=ALU.mult, op1=ALU.add,
                    )
                else:
                    nc.vector.tensor_scalar(
                        OUTt[:, q, :], P[g2][:, q, :], rs, gs_,
                        op0=ALU.mult, op1=ALU.add,
                    )

        for g in range(NG):
            P[g] = pp.tile([R, GS, R], bf16, tag=f"P{g}", name=f"P_{g}")
            for q in range(GS):
                n = GS * g + q
                bank = ps.tile([R, GS, R], f32, tag="ps", name=f"ps_{n}")
                u = bank[:, 0, :]
                nc.tensor.matmul(
                    u,
                    lhsT=IN[g][:, 0, q, :],
                    rhs=IN[g][:, 1, q, :],
                    start=True,
                    stop=False,
                )
                nc.tensor.matmul(
                    u,
                    lhsT=FT[:, 0, n, :],
                    rhs=FT[:, 1, n, :],
                    start=False,
                    stop=True,
                )
                nc.scalar.activation(P[g][:, q, :], u, ACTF.Square)
            nc.vector.tensor_reduce(
                scolt[g][:],
                P[g][:],
                axis=mybir.AxisListType.X,
                op=ALU.add,
            )
            tiny(g)
            if g < 2:
                finals(g, ["dve", "dve", "pool", "pool"])
                nc.sync.dma_start(
                    yd[:, GS * g : GS * (g + 1), :], OUTT[g][:]
                )

        # groups 2,3: finals after ALL squares so ACT can take a lane
        # without blocking the square pipeline (in-order ACT queue)
        finals(2, ["dve", "act", "pool", "pool"])
        nc.sync.dma_start(yd[:, 8:12, :], OUTT[2][:])
        finals(3, ["dve", "act", "pool", "pool"])
        nc.sync.dma_start(yd[:, 12:14, :], OUTT[3][:, 0:2, :])
        nc.gpsimd.dma_start(yd[:, 14:NP, :], OUTT[3][:, 2:GS, :])

    nc.compile()
    return nc


def _build_nc_general(key):
    """Exp/exp fallback (correct for any parameters); key carries per-kernel
    (mode, sc, h, g, w)."""
    from contextlib import ExitStack

    import concourse.bacc as bacc
    import concourse.tile as tile
    from concourse import mybir

    f32 = mybir.dt.float32
    bf16 = mybir.dt.bfloat16
    f8 = mybir.dt.float8e4
    ALU = mybir.AluOpType
    ACTF = mybir.ActivationFunctionType
    mld = _mld()

    KS, per_k = key
    KS = list(KS)
    per_k = dict(zip(KS, per_k))

    nc = bacc.Bacc(
        "TRN2",
        target_bir_lowering=False,
        debug=False,
        enable_asserts=False,
        num_devices=NCORES,
    )
    ATd = {
        k: nc.dram_tensor(f"at{k}", [F, NP, R], f8, kind="ExternalInput").ap()
        for k in KS
    }
    BTd = nc.dram_tensor("bt", [F, NP, R], f8, kind="ExternalInput").ap()
    CBd = nc.dram_tensor("cb", [1, NP, R], bf16, kind="ExternalInput").ap()
    BIASd = {
        k: nc.dram_tensor(f"bias{k}", [R, NP], f32, kind="ExternalInput").ap()
        for k in KS
    }
    Yd = nc.dram_tensor("y", [R, NP, R], f32, kind="ExternalOutput").ap()
    onesd = nc.inline_tensor(
        np.ones((1, R), dtype=mld.bfloat16), name="ones1"
    ).ap()

    with ExitStack() as ctx:
        tc = ctx.enter_context(tile.TileContext(nc))
        singles = ctx.enter_context(tc.tile_pool(name="singles", bufs=1))
        inp = ctx.enter_context(tc.tile_pool(name="inp", bufs=2 * NG))
        pp = ctx.enter_context(tc.tile_pool(name="pp", bufs=3))
        cols = ctx.enter_context(tc.tile_pool(name="cols", bufs=2 * NG))
        ps = ctx.enter_context(tc.tile_pool(name="ps", bufs=8, space="PSUM"))

        ones = singles.tile([1, R], bf16)
        nc.sync.dma_start(ones[:], onesd)
        CBt = singles.tile([1, NP, R], bf16)
        nc.sync.dma_start(CBt[:], CBd)
        BIASt = {
            k: singles.tile([R, NP], f32, name=f"biast{k}") for k in KS
        }
        for k in KS:
            nc.sync.dma_start(BIASt[k][:], BIASd[k])

        AT = {}
        BT = {}
        for g in range(NG):
            s = slice(GS * g, GS * (g + 1))
            for k in KS:
                AT[(k, g)] = inp.tile(
                    [F, GS, R], f8, tag=f"at{k}{g % 2}", name=f"at{k}_{g}"
                )
                nc.sync.dma_start(AT[(k, g)][:], ATd[k][:, s, :])
            BT[g] = inp.tile([F, GS, R], f8, tag=f"bt{g % 2}", name=f"bt_{g}")
            nc.scalar.dma_start(BT[g][:], BTd[:, s, :])

        OUTacc = singles.tile([R, NP, R], f32)

        for g in range(NG):
            s = slice(GS * g, GS * (g + 1))
            for ki, k in enumerate(KS):
                mode, sc, h, gq, wkk = per_k[k]
                pst = ps.tile([R, GS, R], f32, tag="ps")
                for q in range(GS):
                    nc.tensor.matmul(
                        pst[:, q, :],
                        lhsT=AT[(k, g)][:, q, :],
                        rhs=BT[g][:, q, :],
                        start=(q == 0),
                        stop=False,
                    )
                nc.tensor.matmul(
                    pst[:, :, :],
                    lhsT=ones[:],
                    rhs=CBt[:, s, :],
                    start=False,
                    stop=True,
                )
                scol = cols.tile([R, GS], f32, tag="scol")
                KV = pp.tile([R, GS, R], f32, tag="KV")
                E = pp.tile([R, GS, R], f32, tag="E")
                for q in range(GS):
                    n = GS * g + q
                    nc.scalar.activation(
                        KV[:, q, :],
                        pst[:, q, :],
                        ACTF.Exp,
                        bias=BIASt[k][:, n : n + 1],
                        scale=sc,
                    )
                    nc.scalar.activation(
                        E[:, q, :],
                        KV[:, q, :],
                        ACTF.Exp,
                        accum_out=scol[:, q : q + 1],
                    )
                rcol = cols.tile([R, GS], f32, tag="rcol")
                nc.vector.reciprocal_approx_fast(rcol[:], scol[:])
                if wkk != 1.0:
                    nc.vector.tensor_scalar(
                        rcol[:], rcol[:], float(wkk), None, op0=ALU.mult
                    )
                for q in range(GS):
                    n = GS * g + q
                    if ki == 0:
                        nc.vector.tensor_scalar(
                            OUTacc[:, n, :],
                            E[:, q, :],
                            rcol[:, q : q + 1],
                            None,
                            op0=ALU.mult,
                        )
                    else:
                        nc.vector.scalar_tensor_tensor(
                            OUTacc[:, n, :],
                            E[:, q, :],
                            rcol[:, q : q + 1],
                            OUTacc[:, n, :],
                            op0=ALU.mult,
                            op1=ALU.add,
                        )
            eng = nc.sync if g % 2 == 0 else nc.scalar
            eng.dma_start(Yd[:, s, :], OUTacc[:, s, :])

    nc.compile()
    return nc


_CACHE = {}


def run(x1, x2, sigmas, means, sigma_params, trace=False, **rk):
    from concourse.bass_utils import run_bass_kernel_spmd

    x1 = np.ascontiguousarray(x1, dtype=np.float32)
    x2 = np.ascontiguousarray(x2, dtype=np.float32)
    plan = _plan(x1, x2, sigmas, means, sigma_params)
    KS = plan["KS"]

    if plan["fast"]:
        k = KS[0]
        gq = plan["g"][k] / (plan["sc"][k] ** 2)
        key = ("fast", float(gq))
        if key not in _CACHE:
            _CACHE[key] = _build_nc_fast(float(gq))
        nc = _CACHE[key]
        in_maps = [_core_inputs_fast(plan, c) for c in range(NCORES)]
        res = run_bass_kernel_spmd(
            nc, in_maps, core_ids=list(range(NCORES)), trace=trace, **rk
        )
        out = np.concatenate(
            [
                (
                    (np.asarray(r["y"]).astype(np.float32) + 1.0)
                    * np.float32(1.0 / R)
                ).transpose(1, 0, 2)
                for r in res.results
            ],
            axis=0,
        )
        return out, res

    key = (
        tuple(KS),
        tuple(
            (plan["mode"][k], plan["sc"][k], plan["h"][k], plan["g"][k],
             plan["w"][k])
            for k in KS
        ),
    )
    if key not in _CACHE:
        _CACHE[key] = _build_nc_general(key)
    nc = _CACHE[key]
    in_maps = []
    for c in range(NCORES):
        s = slice(c * NP, (c + 1) * NP)
        m = {
            "bt": np.ascontiguousarray(plan["BT"][:, s, :]),
            "cb": np.ascontiguousarray(
                plan["colB"][s].astype(np.float32).astype(_mld().bfloat16)
            )[None],
        }
        for k in KS:
            m[f"at{k}"] = np.ascontiguousarray(plan["AT"][k][:, s, :])
            bias = plan["sc"][k] * plan["rowA"][k][s]  # [NP, R]
            m[f"bias{k}"] = np.ascontiguousarray(
                bias.astype(np.float32).transpose()
            )
        in_maps.append(m)
    res = run_bass_kernel_spmd(
        nc, in_maps, core_ids=list(range(NCORES)), trace=trace, **rk
    )
    out = np.concatenate(
        [np.asarray(r["y"]).astype(np.float32).transpose(1, 0, 2)
         for r in res.results],
        axis=0,
    )
    return out, res


def kernel(x1, x2, sigmas, means, sigma_params):
    out, _ = run(x1, x2, sigmas, means, sigma_params, trace=False)
    return out
